# revision 19
# baseline (speedup 1.0000x reference)
"""Trainium2 Bass kernel for the ContinuousSSM block.

Math summary (derived from the reference):
  The "fixed-point evolution" loop never trips its convergence gate for
  standard-scale inputs (diff_t >= ~1e-2 >> THRESH=1e-4 for all 10 steps),
  so it is exactly the closed form
      y_h = Bx * (1 - A_bar * G^9) / (1 - A_bar),   G = (1 + A_bar)/2
  with A_bar = exp(dt * A), A[d,n] = -exp(A_log)[d,n] (d-independent),
  Bx = (dt*x_inner) outer Bm, and y[l,d] = sum_n y_h * Cm[l,n] + D[d]*x_inner.
  With wc = Bm*Cm and G_n(r) = dt(r)*F_n(dt(r)) (dt = 0.1*softplus(r),
  F_n the closed form above), this collapses to
      y[l,d] = x_i[l,d] * ( sum_j Gam[l,j] * r[l,d]^j + D[d] ),
  Gam = wc @ beta, where beta[:,j] are per-state polynomial fits of G_n over
  r in [-1,1] (|r| <~ 0.05 in practice; clamped to +-1.25 on device).

Sharding: data-parallel over seq_len: 8 cores x 32 positions (+3 halo for
the causal conv), parameters replicated (collectives have a ~20us floor).

Implementation notes:
  - all weights host-pre-arranged to per-partition-contiguous [128, ...]
    layouts; big ones split into ~256KB DMAs across queues
  - LN gain/bias folded into W_in on the host (bias term enters as a
    per-partition scalar on the transposed xz)
  - rstd for both layernorms via bit-trick + Newton rsqrt on DVE, silu
    native, gelu via tanh -> single ACT table set, loaded during startup
  - tensor_scalar-family instructions carry only ONE sync-wait slot:
    every such op is arranged to have at most one foreign-semaphore dep
"""

import numpy as np

import concourse.bass as bass
import concourse.bacc as bacc_mod
import concourse.tile as tile
from concourse import mybir
from concourse import bass_utils

F32 = mybir.dt.float32
F16 = mybir.dt.float16
BF16 = mybir.dt.bfloat16
I32 = mybir.dt.int32
AF = mybir.ActivationFunctionType
OP = mybir.AluOpType

# ---- problem constants (hardcoded per contract) ----
B_SZ, L, DM = 1, 256, 512
DI, DS, DCONV = 1024, 64, 4
DT_BASE, MAX_STEPS = 0.1, 10
NCORES = 8
SH = L // NCORES            # 32 positions per core
HALO = DCONV - 1            # 3
LH = SH + HALO              # 35
NKIN = DM // 128            # 4
NCI = DI // 128             # 8
DH = 256
NCH = DH // 128             # 2
JDEG = 6
JP1 = JDEG + 1
RCLAMP = 1.25
EPS = 1e-5
QMAGIC = 0x5F3759DF

# ---- precision config ----
BIG_DT, BIG_NP = F16, np.float16   # W_in / W_out matmuls
TRANS_DT = BF16                    # (g,l) pack/unpack transposes

# smalls layout (columns of the [128, NSMALL] fp32 constant block)
CW0 = 0                     # conv_w: col 4*c+j
CB0 = 32                    # conv_b
DD0 = 40                    # D
DB2_0 = 48                  # dt_b2
DB1_0 = 56                  # dt_b1 (2 cols)
BWX0 = 58                   # (ln_in_b @ W_in)[:DI]
BWZ0 = 66                   # (ln_in_b @ W_in)[DI:]
NSMALL = 74

_CACHE = {}


def _fit_beta(A_log: np.ndarray) -> np.ndarray:
    a = np.exp(A_log.astype(np.float64))
    a = a[0] if a.ndim == 2 else a
    k = np.arange(400)
    pts = np.cos(np.pi * (k + 0.5) / 400)
    dtp = np.log1p(np.exp(pts)) * DT_BASE
    M = np.exp(-a[None, :] * dtp[:, None])
    G = 0.5 * (1.0 + M)
    Fv = (1.0 - M * G ** (MAX_STEPS - 1)) / (1.0 - M)
    Gv = dtp[:, None] * Fv
    V = pts[:, None] ** np.arange(JP1)
    beta, *_ = np.linalg.lstsq(V, Gv, rcond=None)
    return np.ascontiguousarray(beta.T.astype(np.float32))


def _part_rows(w, nck):
    """[nck*128, F] -> [128, nck, F], row p,c = w[c*128+p]."""
    F = w.shape[1]
    return np.ascontiguousarray(w.reshape(nck, 128, F).transpose(1, 0, 2))


def _nr_rsqrt(nc, work, v_ap, p, name):
    """rstd = 1/sqrt(v + EPS) on DVE only (quake seed + 3 Newton steps)."""
    ve = work.tile([p, 1], F32, name=f"{name}_ve")
    nc.vector.tensor_scalar_add(ve, v_ap, EPS)
    iv = work.tile([p, 1], I32, name=f"{name}_iv")
    nc.vector.tensor_scalar(out=iv, in0=ve.bitcast(I32), scalar1=1,
                            scalar2=None, op0=OP.logical_shift_right)
    nc.vector.tensor_scalar(out=iv, in0=iv, scalar1=-1, scalar2=QMAGIC,
                            op0=OP.mult, op1=OP.add)
    y = work.tile([p, 1], F32, name=f"{name}_y")
    nc.vector.tensor_copy(out=y, in_=iv.bitcast(F32))
    t = work.tile([p, 1], F32, name=f"{name}_t")
    for _ in range(3):
        nc.vector.tensor_mul(t, y, y)
        nc.vector.tensor_mul(t, t, ve)
        nc.vector.tensor_scalar(out=t, in0=t, scalar1=-0.5, scalar2=1.5,
                                op0=OP.mult, op1=OP.add)
        nc.vector.tensor_mul(y, y, t)
    return y


def _build_nc():
    nc = bacc_mod.Bacc()

    p_x = nc.declare_dram_parameter("x_sh", [LH, DM], F32, isOutput=False)
    p_maskt = nc.declare_dram_parameter("maskt", [1, LH], F32, isOutput=False)
    p_win = nc.declare_dram_parameter("w_in", [128, NKIN, 2 * DI], BIG_DT, isOutput=False)
    p_wout = nc.declare_dram_parameter("w_out", [128, NCI, DM], BIG_DT, isOutput=False)
    p_wb = nc.declare_dram_parameter("w_b", [128, NCI, DS], F16, isOutput=False)
    p_wc = nc.declare_dram_parameter("w_c", [128, NCI, DS], F16, isOutput=False)
    p_dw1 = nc.declare_dram_parameter("dt_w1", [128, NCI, DH], F16, isOutput=False)
    p_dw2 = nc.declare_dram_parameter("dt_w2", [128, NCH, DI], F16, isOutput=False)
    p_small = nc.declare_dram_parameter("smalls", [128, NSMALL], F32, isOutput=False)
    p_gout = nc.declare_dram_parameter("ln_out_g", [DM], F32, isOutput=False)
    p_bout = nc.declare_dram_parameter("ln_out_b", [DM], F32, isOutput=False)
    p_beta = nc.declare_dram_parameter("beta", [DS, JP1], F32, isOutput=False)
    p_rep = nc.declare_dram_parameter("rep", [SH, 128], F32, isOutput=False)
    p_id = nc.declare_dram_parameter("ident", [128, 128], F32, isOutput=False)
    p_idt = nc.declare_dram_parameter("ident_t", [128, 128], TRANS_DT, isOutput=False)
    p_out = nc.declare_dram_parameter("out", [SH, DM], F32, isOutput=True)

    def bcast(ap_1d, p):
        return bass.AP(tensor=ap_1d.tensor, offset=ap_1d.offset,
                       ap=[[0, p]] + list(ap_1d.ap))

    from contextlib import ExitStack
    with tile.TileContext(nc) as tc, ExitStack() as ctx:
        cons = ctx.enter_context(tc.tile_pool(name="cons", bufs=1))
        work = ctx.enter_context(tc.tile_pool(name="work", bufs=2))
        psum = ctx.enter_context(tc.tile_pool(name="ps", bufs=3, space="PSUM"))

        # ---- warm the single ACT table set during startup ----
        km = cons.tile([32, 1], F32)
        nc.vector.memset(km, 0.5)
        warm = cons.tile([32, 1], F32)
        nc.scalar.activation(out=warm, in_=km, func=AF.Silu)

        # ---- loads ----
        x_sb = cons.tile([LH, DM], F32)
        nc.sync.dma_start(out=x_sb, in_=p_x[:])
        id_sb = cons.tile([128, 128], F32)
        nc.sync.dma_start(out=id_sb, in_=p_id[:])
        idt_sb = cons.tile([128, 128], TRANS_DT)
        nc.sync.dma_start(out=idt_sb, in_=p_idt[:])
        small_sb = cons.tile([128, NSMALL], F32)
        nc.sync.dma_start(out=small_sb, in_=p_small[:])
        beta_sb = cons.tile([DS, JP1], F32)
        nc.sync.dma_start(out=beta_sb, in_=p_beta[:])
        rep_sb = cons.tile([SH, 128], F32)
        nc.sync.dma_start(out=rep_sb, in_=p_rep[:])
        maskt_rep = cons.tile([128, LH], F32)
        nc.gpsimd.dma_start(out=maskt_rep, in_=bcast(p_maskt[0, :], 128))
        win_sb = cons.tile([128, NKIN, 2 * DI], BIG_DT)
        WSPLIT = 4
        for s in range(WSPLIT):        # s=0,1 cover x_inner columns: first
            for k in range(NKIN):
                w = 2 * DI // WSPLIT
                nc.sync.dma_start(out=win_sb[:, k, s * w:(s + 1) * w],
                                  in_=p_win[:, k, s * w:(s + 1) * w])
        wb_sb = cons.tile([128, NCI, DS], F16)
        nc.sync.dma_start(out=wb_sb, in_=p_wb[:])
        wc_sb = cons.tile([128, NCI, DS], F16)
        nc.sync.dma_start(out=wc_sb, in_=p_wc[:])
        dw1_sb = cons.tile([128, NCI, DH], F16)
        for h in range(2):
            nc.sync.dma_start(out=dw1_sb[:, 4 * h:4 * h + 4, :],
                              in_=p_dw1[:, 4 * h:4 * h + 4, :])
        dw2_sb = cons.tile([128, NCH, DI], F16)
        for k in range(NCH):
            nc.sync.dma_start(out=dw2_sb[:, k, :], in_=p_dw2[:, k, :])
        wout_sb = cons.tile([128, NCI, DM], BIG_DT)
        for h in range(NCI):
            nc.sync.dma_start(out=wout_sb[:, h, :], in_=p_wout[:, h, :])
        gout_rep = cons.tile([SH, DM], F32)
        nc.gpsimd.dma_start(out=gout_rep, in_=bcast(p_gout[:], SH))
        bout_rep = cons.tile([SH, DM], F32)
        nc.gpsimd.dma_start(out=bout_rep, in_=bcast(p_bout[:], SH))
        xres_sb = cons.tile([SH, DM], F32)
        nc.sync.dma_start(out=xres_sb, in_=p_x[HALO:, :])

        # ---- 1. input layernorm (l on partitions) ----
        st1 = work.tile([LH, 6], F32)
        nc.vector.bn_stats(out=st1, in_=x_sb)
        mv1 = work.tile([LH, 2], F32)
        nc.vector.bn_aggr(out=mv1, in_=st1)
        rstd1 = _nr_rsqrt(nc, work, mv1[:, 1:2], LH, "r1")
        xhat = work.tile([LH, DM], F32)
        nc.vector.tensor_scalar(out=xhat, in0=x_sb, scalar1=mv1[:, 0:1],
                                scalar2=rstd1, op0=OP.subtract, op1=OP.mult)
        # observers: make DVE see the smalls + mask DMA queues once, so later
        # tensor_scalar ops only carry their PE wait
        sm_obs = work.tile([128, 1], F32)
        nc.vector.tensor_scalar_mul(sm_obs, small_sb[:, 0:1], 1.0)
        mask_obs = work.tile([128, LH], F32)
        nc.vector.tensor_scalar_mul(mask_obs, maskt_rep, 1.0)

        # ---- 2. transpose xhat -> xnT [128, NKIN, LH] ----
        xnT = work.tile([128, NKIN, LH], BIG_DT)
        for k in range(NKIN):
            ps_t = psum.tile([128, LH], F32, tag="mm")
            nc.tensor.matmul(ps_t, xhat[:, k * 128:(k + 1) * 128],
                             id_sb[:LH, :LH], is_transpose=True,
                             start=True, stop=True)
            nc.vector.tensor_copy(out=xnT[:, k, :], in_=ps_t)

        # ---- 3. xz = xhat @ (g*W_in) [+ b@W_in via bias] ----
        xr = []
        zsil = []
        for m in range(2 * NCI):
            n0 = 0 if m < NCI else HALO
            ps_xz = psum.tile([128, LH - n0], F32, tag="mm")
            for k in range(NKIN):
                nc.tensor.matmul(ps_xz, win_sb[:, k, m * 128:(m + 1) * 128],
                                 xnT[:, k, n0:],
                                 start=(k == 0), stop=(k == NKIN - 1))
            if m < NCI:
                t = work.tile([128, LH], F32, tag="xr", bufs=NCI)
                nc.vector.scalar_tensor_tensor(
                    out=t, in0=ps_xz, scalar=small_sb[:, BWX0 + m:BWX0 + m + 1],
                    in1=mask_obs, op0=OP.add, op1=OP.mult)
                xr.append(t)
            else:
                c = m - NCI
                t = work.tile([128, SH], F32, tag="zsil", bufs=NCI)
                nc.scalar.activation(out=t, in_=ps_xz, func=AF.Silu,
                                     bias=small_sb[:, BWZ0 + c:BWZ0 + c + 1])
                zsil.append(t)

        # ---- 4. depthwise causal conv + silu (-> fp16) ----
        xiT16 = []
        for c in range(NCI):
            acc = work.tile([128, SH], F32, tag="cacc")
            nc.vector.tensor_scalar_mul(acc, xr[c][:, 0:SH],
                                        small_sb[:, CW0 + 4 * c:CW0 + 4 * c + 1])
            for j in range(1, DCONV):
                nc.vector.scalar_tensor_tensor(
                    out=acc, in0=xr[c][:, j:SH + j],
                    scalar=small_sb[:, CW0 + 4 * c + j:CW0 + 4 * c + j + 1],
                    in1=acc, op0=OP.mult, op1=OP.add)
            xi16 = work.tile([128, SH], F16, tag="xi16", bufs=NCI)
            nc.scalar.activation(out=xi16, in_=acc, func=AF.Silu,
                                 bias=small_sb[:, CB0 + c:CB0 + c + 1])
            xiT16.append(xi16)

        # ---- 5. Bm/Cm/wc and Gamma ----
        ps_bm = psum.tile([DS, SH], F32, tag="acc", bufs=2)
        for c in range(NCI):
            nc.tensor.matmul(ps_bm, wb_sb[:, c, :], xiT16[c],
                             start=(c == 0), stop=(c == NCI - 1))
        ps_cm = psum.tile([DS, SH], F32, tag="acc", bufs=2)
        for c in range(NCI):
            nc.tensor.matmul(ps_cm, wc_sb[:, c, :], xiT16[c],
                             start=(c == 0), stop=(c == NCI - 1))
        bm_sb = work.tile([DS, SH], F32)
        nc.vector.tensor_copy(out=bm_sb, in_=ps_bm)
        wcp_sb = work.tile([DS, SH], F32)
        nc.vector.tensor_mul(wcp_sb, ps_cm, bm_sb)

        ps_gam = psum.tile([SH, JP1], F32, tag="acc", bufs=2)
        nc.tensor.matmul(ps_gam, wcp_sb, beta_sb, start=True, stop=True)
        gam_sb = work.tile([SH, JP1], F32)
        nc.vector.tensor_copy(out=gam_sb, in_=ps_gam)
        ps_g128 = psum.tile([128, JP1], F32, tag="acc", bufs=2)
        nc.tensor.matmul(ps_g128, rep_sb, gam_sb, start=True, stop=True)
        g128 = work.tile([128, JP1], F32)
        nc.vector.tensor_copy(out=g128, in_=ps_g128)

        # ---- 6. dt MLP -> r (pre-softplus; gelu via tanh, its x0.5 factor
        # folded into dt_w2 host-side) ----
        gel16 = []
        for mc in range(NCH):
            ps_g1 = psum.tile([128, SH], F32, tag="mm")
            for c in range(NCI):
                nc.tensor.matmul(ps_g1, dw1_sb[:, c, mc * 128:(mc + 1) * 128],
                                 xiT16[c], start=(c == 0), stop=(c == NCI - 1))
            x2 = work.tile([128, SH], F32, tag="gx2")
            nc.scalar.activation(out=x2, in_=ps_g1, func=AF.Square,
                                 bias=small_sb[:, DB1_0 + mc:DB1_0 + mc + 1])
            g1b = work.tile([128, SH], F32, tag="g1b", bufs=NCH)
            nc.scalar.activation(out=g1b, in_=ps_g1, func=AF.Identity,
                                 bias=small_sb[:, DB1_0 + mc:DB1_0 + mc + 1])
            t1s = work.tile([128, SH], F32, tag="gt1")
            nc.vector.tensor_scalar(out=t1s, in0=x2, scalar1=0.03567740814,
                                    scalar2=0.79788456080, op0=OP.mult, op1=OP.add)
            arg = work.tile([128, SH], F32, tag="garg")
            nc.vector.tensor_mul(arg, t1s, g1b)
            th = work.tile([128, SH], F32, tag="gth")
            nc.scalar.activation(out=th, in_=arg, func=AF.Tanh)
            g = work.tile([128, SH], F16, tag="gel", bufs=NCH)
            nc.vector.scalar_tensor_tensor(out=g, in0=th, scalar=1.0,
                                           in1=g1b, op0=OP.add, op1=OP.mult)
            gel16.append(g)
        u_sb = []
        for c in range(NCI):
            ps_r = psum.tile([128, SH], F32, tag="mm")
            for k in range(NCH):
                nc.tensor.matmul(ps_r, dw2_sb[:, k, c * 128:(c + 1) * 128],
                                 gel16[k], start=(k == 0), stop=(k == NCH - 1))
            u = work.tile([128, SH], TRANS_DT, tag="u", bufs=NCI)
            nc.scalar.activation(out=u, in_=ps_r, func=AF.Identity,
                                 bias=small_sb[:, DB2_0 + c:DB2_0 + c + 1])
            u_sb.append(u)

        # ---- 7. pack r to (group,l) layout ----
        ps_u = psum.tile([128, 2 * 128], F32, tag="pack", bufs=1)
        for c in range(NCI):
            g, hf = c // 2, c % 2
            nc.tensor.matmul(ps_u[g * 32:(g + 1) * 32, hf * 128:(hf + 1) * 128],
                             u_sb[c], idt_sb,
                             tile_position=(0, g * 32), start=True, stop=True)
        ugl = work.tile([128, 256], F32)
        nc.vector.tensor_scalar(out=ugl, in0=ps_u, scalar1=RCLAMP,
                                scalar2=-RCLAMP, op0=OP.min, op1=OP.max)

        # ---- 8. Horner ----
        wh = work.tile([128, 256], F32)
        nc.vector.tensor_scalar_mul(wh, ugl, g128[:, JDEG:JDEG + 1])
        for k in range(JDEG - 1, 0, -1):
            nc.vector.scalar_tensor_tensor(out=wh, in0=wh,
                                           scalar=g128[:, k:k + 1], in1=ugl,
                                           op0=OP.add, op1=OP.mult)
        t1 = work.tile([128, 256], TRANS_DT)
        nc.vector.tensor_scalar_add(t1, wh, g128[:, 0:1])

        # ---- 9. unpack, gate, W_out ----
        yg = []
        for c in range(NCI):
            g, hf = c // 2, c % 2
            ps_ts = psum.tile([128, SH], F32, tag="mm")
            nc.tensor.matmul(ps_ts, t1[g * 32:(g + 1) * 32, hf * 128:(hf + 1) * 128],
                             idt_sb[g * 32:(g + 1) * 32, g * 32:(g + 1) * 32],
                             tile_position=(g * 32, 0),
                             start=True, stop=True)
            y = work.tile([128, SH], F32, tag="y", bufs=NCI)
            nc.scalar.activation(out=y, in_=ps_ts, func=AF.Identity,
                                 bias=small_sb[:, DD0 + c:DD0 + c + 1])
            nc.vector.tensor_mul(y, y, xiT16[c])
            y2 = work.tile([128, SH], BIG_DT, tag="y2", bufs=NCI)
            nc.vector.tensor_mul(y2, y, zsil[c])
            yg.append(y2)

        oT = []
        for m in range(NKIN):
            ps_o = psum.tile([128, SH], F32, tag="mm")
            for c in range(NCI):
                nc.tensor.matmul(ps_o, wout_sb[:, c, m * 128:(m + 1) * 128],
                                 yg[c], start=(c == 0), stop=(c == NCI - 1))
            t = work.tile([128, SH], F32, tag="oT", bufs=NKIN)
            nc.vector.tensor_copy(out=t, in_=ps_o)
            oT.append(t)

        # ---- 10. final transpose + layernorm + residual ----
        ps_fin = psum.tile([SH, DM], F32, tag="fin", bufs=1)
        for m in range(NKIN):
            nc.tensor.matmul(ps_fin[:, m * 128:(m + 1) * 128], oT[m],
                             id_sb, is_transpose=True, start=True, stop=True)
        st2 = work.tile([SH, 6], F32)
        nc.vector.bn_stats(out=st2, in_=ps_fin)
        mv2 = work.tile([SH, 2], F32)
        nc.vector.bn_aggr(out=mv2, in_=st2)
        rstd2 = _nr_rsqrt(nc, work, mv2[:, 1:2], SH, "r2")
        xhat2 = work.tile([SH, DM], F32)
        nc.vector.tensor_scalar(out=xhat2, in0=ps_fin, scalar1=mv2[:, 0:1],
                                scalar2=rstd2, op0=OP.subtract, op1=OP.mult)
        rb = work.tile([SH, DM], F32)
        nc.vector.tensor_add(rb, bout_rep, xres_sb)
        outf = work.tile([SH, DM], F32)
        nc.vector.tensor_mul(outf, xhat2, gout_rep)
        nc.vector.tensor_add(outf, outf, rb)
        nc.sync.dma_start(out=p_out[:], in_=outf)

    nc.finalize()
    return nc


def _make_in_maps(inputs):
    x = np.asarray(inputs["x"], np.float32)
    A_log = np.asarray(inputs["A_log"], np.float32)
    beta = _fit_beta(A_log)
    rep = np.zeros((SH, 128), np.float32)
    rep[np.arange(128) % SH, np.arange(128)] = 1.0
    ident = np.eye(128, dtype=np.float32)

    if TRANS_DT == F32:
        tnp = np.float32
    elif TRANS_DT == F16:
        tnp = np.float16
    else:
        import ml_dtypes
        tnp = ml_dtypes.bfloat16

    W_in = np.asarray(inputs["W_in"], np.float32)
    g_in = np.asarray(inputs["ln_in_g"], np.float32)
    b_in = np.asarray(inputs["ln_in_b"], np.float32)
    W_in_g = g_in[:, None] * W_in
    bw = (b_in @ W_in).astype(np.float32)

    smalls = np.zeros((128, NSMALL), np.float32)
    cw = np.asarray(inputs["conv_w"], np.float32)[:, 0, :].reshape(NCI, 128, DCONV)
    for c in range(NCI):
        smalls[:, CW0 + 4 * c:CW0 + 4 * c + 4] = cw[c]
    smalls[:, CB0:CB0 + NCI] = np.asarray(inputs["conv_b"], np.float32).reshape(NCI, 128).T
    smalls[:, DD0:DD0 + NCI] = np.asarray(inputs["D"], np.float32).reshape(NCI, 128).T
    smalls[:, DB2_0:DB2_0 + NCI] = np.asarray(inputs["dt_b2"], np.float32).reshape(NCI, 128).T
    smalls[:, DB1_0:DB1_0 + NCH] = np.asarray(inputs["dt_b1"], np.float32).reshape(NCH, 128).T
    smalls[:, BWX0:BWX0 + NCI] = bw[:DI].reshape(NCI, 128).T
    smalls[:, BWZ0:BWZ0 + NCI] = bw[DI:].reshape(NCI, 128).T

    shared = {
        "w_in": _part_rows(W_in_g, NKIN).astype(BIG_NP),
        "w_out": _part_rows(np.asarray(inputs["W_out"], np.float32), NCI).astype(BIG_NP),
        "w_b": _part_rows(np.asarray(inputs["W_B"], np.float32), NCI).astype(np.float16),
        "w_c": _part_rows(np.asarray(inputs["W_C"], np.float32), NCI).astype(np.float16),
        "dt_w1": _part_rows(np.asarray(inputs["dt_w1"], np.float32), NCI).astype(np.float16),
        "dt_w2": _part_rows(0.5 * np.asarray(inputs["dt_w2"], np.float32), NCH).astype(np.float16),
        "smalls": smalls,
        "ln_out_g": np.asarray(inputs["ln_out_g"], np.float32),
        "ln_out_b": np.asarray(inputs["ln_out_b"], np.float32),
        "beta": beta,
        "rep": rep,
        "ident": ident,
        "ident_t": ident.astype(tnp),
    }

    xf = x[0]
    in_maps = []
    for core in range(NCORES):
        lo = core * SH - HALO
        xs = np.zeros((LH, DM), np.float32)
        mskt = np.zeros((1, LH), np.float32)
        valid0 = max(0, -lo)
        xs[valid0:] = xf[lo + valid0: lo + LH]
        mskt[0, valid0:] = 1.0
        in_maps.append({**shared, "x_sh": xs, "maskt": mskt})
    return in_maps


def kernel(**inputs):
    if "nc" not in _CACHE:
        _CACHE["nc"] = _build_nc()
    nc = _CACHE["nc"]
    in_maps = _make_in_maps(inputs)
    res = bass_utils.run_bass_kernel_spmd(nc, in_maps, core_ids=list(range(NCORES)))
    out = np.concatenate([res.results[i]["out"] for i in range(NCORES)], axis=0)
    return out.reshape(1, L, DM).astype(np.float32)


# revision 20
# speedup vs baseline: 1.0141x; 1.0141x over previous
"""Trainium2 Bass kernel for the ContinuousSSM block.

Math summary (derived from the reference):
  The "fixed-point evolution" loop never trips its convergence gate for
  standard-scale inputs (diff_t >= ~1e-2 >> THRESH=1e-4 for all 10 steps),
  so it is exactly the closed form
      y_h = Bx * (1 - A_bar * G^9) / (1 - A_bar),   G = (1 + A_bar)/2
  with A_bar = exp(dt * A), A[d,n] = -exp(A_log)[d,n] (d-independent),
  Bx = (dt*x_inner) outer Bm, and y[l,d] = sum_n y_h * Cm[l,n] + D[d]*x_inner.
  With wc = Bm*Cm and G_n(r) = dt(r)*F_n(dt(r)) (dt = 0.1*softplus(r),
  F_n the closed form above), this collapses to
      y[l,d] = x_i[l,d] * ( sum_j Gam[l,j] * r[l,d]^j + D[d] ),
  Gam = wc @ beta, where beta[:,j] are per-state polynomial fits of G_n over
  r in [-1,1] (|r| <~ 0.05 in practice; clamped to +-1.25 on device).

Sharding: data-parallel over seq_len: 8 cores x 32 positions (+3 halo for
the causal conv), parameters replicated (collectives have a ~20us floor).

Implementation notes:
  - all weights host-pre-arranged to per-partition-contiguous [128, ...]
    layouts; big ones split into ~256KB DMAs across queues
  - LN gain/bias folded into W_in on the host (bias term enters as a
    per-partition scalar on the transposed xz)
  - rstd for both layernorms via bit-trick + Newton rsqrt on DVE, silu
    native, gelu via tanh -> single ACT table set, loaded during startup
  - tensor_scalar-family instructions carry only ONE sync-wait slot:
    every such op is arranged to have at most one foreign-semaphore dep
"""

import numpy as np

import concourse.bass as bass
import concourse.bacc as bacc_mod
import concourse.tile as tile
from concourse import mybir
from concourse import bass_utils

F32 = mybir.dt.float32
F16 = mybir.dt.float16
BF16 = mybir.dt.bfloat16
I32 = mybir.dt.int32
AF = mybir.ActivationFunctionType
OP = mybir.AluOpType

# ---- problem constants (hardcoded per contract) ----
B_SZ, L, DM = 1, 256, 512
DI, DS, DCONV = 1024, 64, 4
DT_BASE, MAX_STEPS = 0.1, 10
NCORES = 8
SH = L // NCORES            # 32 positions per core
HALO = DCONV - 1            # 3
LH = SH + HALO              # 35
NKIN = DM // 128            # 4
NCI = DI // 128             # 8
DH = 256
NCH = DH // 128             # 2
JDEG = 6
JP1 = JDEG + 1
RCLAMP = 1.25
EPS = 1e-5
QMAGIC = 0x5F3759DF

# ---- precision config ----
BIG_DT, BIG_NP = F16, np.float16   # W_in / W_out matmuls
TRANS_DT = BF16                    # (g,l) pack/unpack transposes

# smalls layout (columns of the [128, NSMALL] fp32 constant block)
CW0 = 0                     # conv_w: col 4*c+j
CB0 = 32                    # conv_b
DD0 = 40                    # D
DB2_0 = 48                  # dt_b2
DB1_0 = 56                  # dt_b1 (2 cols)
BWX0 = 58                   # (ln_in_b @ W_in)[:DI]
BWZ0 = 66                   # (ln_in_b @ W_in)[DI:]
NSMALL = 74

_CACHE = {}


def _fit_beta(A_log: np.ndarray) -> np.ndarray:
    a = np.exp(A_log.astype(np.float64))
    a = a[0] if a.ndim == 2 else a
    k = np.arange(400)
    pts = np.cos(np.pi * (k + 0.5) / 400)
    dtp = np.log1p(np.exp(pts)) * DT_BASE
    M = np.exp(-a[None, :] * dtp[:, None])
    G = 0.5 * (1.0 + M)
    Fv = (1.0 - M * G ** (MAX_STEPS - 1)) / (1.0 - M)
    Gv = dtp[:, None] * Fv
    V = pts[:, None] ** np.arange(JP1)
    beta, *_ = np.linalg.lstsq(V, Gv, rcond=None)
    return np.ascontiguousarray(beta.T.astype(np.float32))


def _part_rows(w, nck):
    """[nck*128, F] -> [128, nck, F], row p,c = w[c*128+p]."""
    F = w.shape[1]
    return np.ascontiguousarray(w.reshape(nck, 128, F).transpose(1, 0, 2))


def _nr_rsqrt(nc, work, v_ap, p, name):
    """rstd = 1/sqrt(v + EPS) on DVE only (quake seed + 3 Newton steps)."""
    ve = work.tile([p, 1], F32, name=f"{name}_ve")
    nc.vector.tensor_scalar_add(ve, v_ap, EPS)
    iv = work.tile([p, 1], I32, name=f"{name}_iv")
    nc.vector.tensor_scalar(out=iv, in0=ve.bitcast(I32), scalar1=1,
                            scalar2=None, op0=OP.logical_shift_right)
    nc.vector.tensor_scalar(out=iv, in0=iv, scalar1=-1, scalar2=QMAGIC,
                            op0=OP.mult, op1=OP.add)
    y = work.tile([p, 1], F32, name=f"{name}_y")
    nc.vector.tensor_copy(out=y, in_=iv.bitcast(F32))
    t = work.tile([p, 1], F32, name=f"{name}_t")
    for _ in range(3):
        nc.vector.tensor_mul(t, y, y)
        nc.vector.tensor_mul(t, t, ve)
        nc.vector.tensor_scalar(out=t, in0=t, scalar1=-0.5, scalar2=1.5,
                                op0=OP.mult, op1=OP.add)
        nc.vector.tensor_mul(y, y, t)
    return y


def _build_nc():
    nc = bacc_mod.Bacc()

    p_x = nc.declare_dram_parameter("x_sh", [LH, DM], F32, isOutput=False)
    p_reps = nc.declare_dram_parameter("reps", [128, LH + 2 * DM], F32, isOutput=False)
    p_win = nc.declare_dram_parameter("w_in", [128, NKIN, 2 * DI], BIG_DT, isOutput=False)
    p_wout = nc.declare_dram_parameter("w_out", [128, NCI, DM], BIG_DT, isOutput=False)
    p_wb = nc.declare_dram_parameter("w_b", [128, NCI, DS], F16, isOutput=False)
    p_wc = nc.declare_dram_parameter("w_c", [128, NCI, DS], F16, isOutput=False)
    p_dw1 = nc.declare_dram_parameter("dt_w1", [128, NCI, DH], F16, isOutput=False)
    p_dw2 = nc.declare_dram_parameter("dt_w2", [128, NCH, DI], F16, isOutput=False)
    p_small = nc.declare_dram_parameter("smalls", [128, NSMALL], F32, isOutput=False)
    p_beta = nc.declare_dram_parameter("beta", [DS, JP1], F32, isOutput=False)
    p_rep = nc.declare_dram_parameter("rep", [SH, 128], F32, isOutput=False)
    p_id = nc.declare_dram_parameter("ident", [128, 128], F32, isOutput=False)
    p_idt = nc.declare_dram_parameter("ident_t", [128, 128], TRANS_DT, isOutput=False)
    p_out = nc.declare_dram_parameter("out", [SH, DM], F32, isOutput=True)

    from contextlib import ExitStack
    with tile.TileContext(nc) as tc, ExitStack() as ctx:
        cons = ctx.enter_context(tc.tile_pool(name="cons", bufs=1))
        work = ctx.enter_context(tc.tile_pool(name="work", bufs=2))
        psum = ctx.enter_context(tc.tile_pool(name="ps", bufs=3, space="PSUM"))

        # ---- warm the single ACT table set during startup ----
        km = cons.tile([32, 1], F32)
        nc.vector.memset(km, 0.5)
        warm = cons.tile([32, 1], F32)
        nc.scalar.activation(out=warm, in_=km, func=AF.Silu)

        # ---- loads ----
        x_sb = cons.tile([LH, DM], F32)
        nc.sync.dma_start(out=x_sb, in_=p_x[:])
        id_sb = cons.tile([128, 128], F32)
        nc.sync.dma_start(out=id_sb, in_=p_id[:])
        idt_sb = cons.tile([128, 128], TRANS_DT)
        nc.sync.dma_start(out=idt_sb, in_=p_idt[:])
        small_sb = cons.tile([128, NSMALL], F32)
        nc.sync.dma_start(out=small_sb, in_=p_small[:])
        beta_sb = cons.tile([DS, JP1], F32)
        nc.sync.dma_start(out=beta_sb, in_=p_beta[:])
        rep_sb = cons.tile([SH, 128], F32)
        nc.sync.dma_start(out=rep_sb, in_=p_rep[:])
        reps_sb = cons.tile([128, LH + 2 * DM], F32)
        nc.sync.dma_start(out=reps_sb, in_=p_reps[:])
        maskt_rep = reps_sb[:, 0:LH]
        gout_rep = reps_sb[0:SH, LH:LH + DM]
        bout_rep = reps_sb[0:SH, LH + DM:LH + 2 * DM]
        win_sb = cons.tile([128, NKIN, 2 * DI], BIG_DT)
        WSPLIT = 4
        for s in range(WSPLIT):        # s=0,1 cover x_inner columns: first
            for k in range(NKIN):
                w = 2 * DI // WSPLIT
                nc.sync.dma_start(out=win_sb[:, k, s * w:(s + 1) * w],
                                  in_=p_win[:, k, s * w:(s + 1) * w])
        wb_sb = cons.tile([128, NCI, DS], F16)
        nc.sync.dma_start(out=wb_sb, in_=p_wb[:])
        wc_sb = cons.tile([128, NCI, DS], F16)
        nc.sync.dma_start(out=wc_sb, in_=p_wc[:])
        dw1_sb = cons.tile([128, NCI, DH], F16)
        for h in range(2):
            nc.sync.dma_start(out=dw1_sb[:, 4 * h:4 * h + 4, :],
                              in_=p_dw1[:, 4 * h:4 * h + 4, :])
        dw2_sb = cons.tile([128, NCH, DI], F16)
        for k in range(NCH):
            nc.sync.dma_start(out=dw2_sb[:, k, :], in_=p_dw2[:, k, :])
        wout_sb = cons.tile([128, NCI, DM], BIG_DT)
        for h in range(NCI):
            nc.sync.dma_start(out=wout_sb[:, h, :], in_=p_wout[:, h, :])
        xres_sb = cons.tile([SH, DM], F32)
        nc.sync.dma_start(out=xres_sb, in_=p_x[HALO:, :])

        # ---- 1. input layernorm (l on partitions) ----
        st1 = work.tile([LH, 6], F32)
        nc.vector.bn_stats(out=st1, in_=x_sb)
        mv1 = work.tile([LH, 2], F32)
        nc.vector.bn_aggr(out=mv1, in_=st1)
        rstd1 = _nr_rsqrt(nc, work, mv1[:, 1:2], LH, "r1")
        xhat = work.tile([LH, DM], F32)
        nc.vector.tensor_scalar(out=xhat, in0=x_sb, scalar1=mv1[:, 0:1],
                                scalar2=rstd1, op0=OP.subtract, op1=OP.mult)
        # observers: make DVE see the smalls + mask DMA queues once, so later
        # tensor_scalar ops only carry their PE wait
        sm_obs = work.tile([128, 1], F32)
        nc.vector.tensor_scalar_mul(sm_obs, small_sb[:, 0:1], 1.0)
        mask_obs = work.tile([128, LH], F32)
        nc.vector.tensor_scalar_mul(mask_obs, maskt_rep, 1.0)

        # ---- 2. transpose xhat -> xnT [128, NKIN, LH] ----
        xnT = work.tile([128, NKIN, LH], BIG_DT)
        for k in range(NKIN):
            ps_t = psum.tile([128, LH], F32, tag="mm")
            nc.tensor.matmul(ps_t, xhat[:, k * 128:(k + 1) * 128],
                             id_sb[:LH, :LH], is_transpose=True,
                             start=True, stop=True)
            nc.vector.tensor_copy(out=xnT[:, k, :], in_=ps_t)

        # ---- 3. xz = xhat @ (g*W_in) [+ b@W_in via bias] ----
        xr = []
        zsil = []
        for m in range(2 * NCI):
            n0 = 0 if m < NCI else HALO
            ps_xz = psum.tile([128, LH - n0], F32, tag="mm")
            for k in range(NKIN):
                nc.tensor.matmul(ps_xz, win_sb[:, k, m * 128:(m + 1) * 128],
                                 xnT[:, k, n0:],
                                 start=(k == 0), stop=(k == NKIN - 1))
            if m < NCI:
                t = work.tile([128, LH], F32, tag="xr", bufs=NCI)
                nc.vector.scalar_tensor_tensor(
                    out=t, in0=ps_xz, scalar=small_sb[:, BWX0 + m:BWX0 + m + 1],
                    in1=mask_obs, op0=OP.add, op1=OP.mult)
                xr.append(t)
            else:
                c = m - NCI
                t = work.tile([128, SH], F32, tag="zsil", bufs=NCI)
                nc.scalar.activation(out=t, in_=ps_xz, func=AF.Silu,
                                     bias=small_sb[:, BWZ0 + c:BWZ0 + c + 1])
                zsil.append(t)

        # ---- 4. depthwise causal conv + silu (-> fp16) ----
        xiT16 = []
        for c in range(NCI):
            acc = work.tile([128, SH], F32, tag="cacc")
            nc.vector.tensor_scalar_mul(acc, xr[c][:, 0:SH],
                                        small_sb[:, CW0 + 4 * c:CW0 + 4 * c + 1])
            for j in range(1, DCONV):
                nc.vector.scalar_tensor_tensor(
                    out=acc, in0=xr[c][:, j:SH + j],
                    scalar=small_sb[:, CW0 + 4 * c + j:CW0 + 4 * c + j + 1],
                    in1=acc, op0=OP.mult, op1=OP.add)
            xi16 = work.tile([128, SH], F16, tag="xi16", bufs=NCI)
            nc.scalar.activation(out=xi16, in_=acc, func=AF.Silu,
                                 bias=small_sb[:, CB0 + c:CB0 + c + 1])
            xiT16.append(xi16)

        # ---- 5. Bm/Cm/wc and Gamma ----
        ps_bm = psum.tile([DS, SH], F32, tag="acc", bufs=2)
        for c in range(NCI):
            nc.tensor.matmul(ps_bm, wb_sb[:, c, :], xiT16[c],
                             start=(c == 0), stop=(c == NCI - 1))
        ps_cm = psum.tile([DS, SH], F32, tag="acc", bufs=2)
        for c in range(NCI):
            nc.tensor.matmul(ps_cm, wc_sb[:, c, :], xiT16[c],
                             start=(c == 0), stop=(c == NCI - 1))
        bm_sb = work.tile([DS, SH], F32)
        nc.vector.tensor_copy(out=bm_sb, in_=ps_bm)
        wcp_sb = work.tile([DS, SH], F32)
        nc.vector.tensor_mul(wcp_sb, ps_cm, bm_sb)

        ps_gam = psum.tile([SH, JP1], F32, tag="acc", bufs=2)
        nc.tensor.matmul(ps_gam, wcp_sb, beta_sb, start=True, stop=True)
        gam_sb = work.tile([SH, JP1], F32)
        nc.vector.tensor_copy(out=gam_sb, in_=ps_gam)
        ps_g128 = psum.tile([128, JP1], F32, tag="acc", bufs=2)
        nc.tensor.matmul(ps_g128, rep_sb, gam_sb, start=True, stop=True)
        g128 = work.tile([128, JP1], F32)
        nc.vector.tensor_copy(out=g128, in_=ps_g128)

        # ---- 6. dt MLP -> r (pre-softplus; gelu via tanh, its x0.5 factor
        # folded into dt_w2 host-side) ----
        gel16 = []
        for mc in range(NCH):
            ps_g1 = psum.tile([128, SH], F32, tag="mm")
            for c in range(NCI):
                nc.tensor.matmul(ps_g1, dw1_sb[:, c, mc * 128:(mc + 1) * 128],
                                 xiT16[c], start=(c == 0), stop=(c == NCI - 1))
            x2 = work.tile([128, SH], F32, tag="gx2")
            nc.scalar.activation(out=x2, in_=ps_g1, func=AF.Square,
                                 bias=small_sb[:, DB1_0 + mc:DB1_0 + mc + 1])
            g1b = work.tile([128, SH], F32, tag="g1b", bufs=NCH)
            nc.scalar.activation(out=g1b, in_=ps_g1, func=AF.Identity,
                                 bias=small_sb[:, DB1_0 + mc:DB1_0 + mc + 1])
            t1s = work.tile([128, SH], F32, tag="gt1")
            nc.vector.tensor_scalar(out=t1s, in0=x2, scalar1=0.03567740814,
                                    scalar2=0.79788456080, op0=OP.mult, op1=OP.add)
            arg = work.tile([128, SH], F32, tag="garg")
            nc.vector.tensor_mul(arg, t1s, g1b)
            th = work.tile([128, SH], F32, tag="gth")
            nc.scalar.activation(out=th, in_=arg, func=AF.Tanh)
            g = work.tile([128, SH], F16, tag="gel", bufs=NCH)
            nc.vector.scalar_tensor_tensor(out=g, in0=th, scalar=1.0,
                                           in1=g1b, op0=OP.add, op1=OP.mult)
            gel16.append(g)
        u_sb = []
        for c in range(NCI):
            ps_r = psum.tile([128, SH], F32, tag="mm")
            for k in range(NCH):
                nc.tensor.matmul(ps_r, dw2_sb[:, k, c * 128:(c + 1) * 128],
                                 gel16[k], start=(k == 0), stop=(k == NCH - 1))
            u = work.tile([128, SH], TRANS_DT, tag="u", bufs=NCI)
            nc.scalar.activation(out=u, in_=ps_r, func=AF.Identity,
                                 bias=small_sb[:, DB2_0 + c:DB2_0 + c + 1])
            u_sb.append(u)

        # ---- 7. pack r to (group,l) layout ----
        ps_u = psum.tile([128, 2 * 128], F32, tag="pack", bufs=1)
        for c in range(NCI):
            g, hf = c // 2, c % 2
            nc.tensor.matmul(ps_u[g * 32:(g + 1) * 32, hf * 128:(hf + 1) * 128],
                             u_sb[c], idt_sb,
                             tile_position=(0, g * 32), start=True, stop=True)
        # ---- 8. Horner (per column-half, so unpack overlaps) ----
        t1 = work.tile([128, 256], TRANS_DT)
        for hf in range(2):
            sl = slice(hf * 128, (hf + 1) * 128)
            ugl = work.tile([128, 128], F32, tag="ugl")
            nc.vector.tensor_scalar(out=ugl, in0=ps_u[:, sl], scalar1=RCLAMP,
                                    scalar2=-RCLAMP, op0=OP.min, op1=OP.max)
            wh = work.tile([128, 128], F32, tag="wh")
            nc.vector.tensor_scalar_mul(wh, ugl, g128[:, JDEG:JDEG + 1])
            for k in range(JDEG - 1, 0, -1):
                nc.vector.scalar_tensor_tensor(out=wh, in0=wh,
                                               scalar=g128[:, k:k + 1], in1=ugl,
                                               op0=OP.add, op1=OP.mult)
            nc.vector.tensor_scalar_add(t1[:, sl], wh, g128[:, 0:1])

        # ---- 9. unpack, gate, W_out (half-0 chunks first: even c) ----
        yg = [None] * NCI
        for c in [0, 2, 4, 6, 1, 3, 5, 7]:
            g, hf = c // 2, c % 2
            ps_ts = psum.tile([128, SH], F32, tag="mm")
            nc.tensor.matmul(ps_ts, t1[g * 32:(g + 1) * 32, hf * 128:(hf + 1) * 128],
                             idt_sb[g * 32:(g + 1) * 32, g * 32:(g + 1) * 32],
                             tile_position=(g * 32, 0),
                             start=True, stop=True)
            y = work.tile([128, SH], F32, tag="y", bufs=NCI)
            nc.scalar.activation(out=y, in_=ps_ts, func=AF.Identity,
                                 bias=small_sb[:, DD0 + c:DD0 + c + 1])
            nc.vector.tensor_mul(y, y, xiT16[c])
            y2 = work.tile([128, SH], BIG_DT, tag="y2", bufs=NCI)
            nc.vector.tensor_mul(y2, y, zsil[c])
            yg[c] = y2

        oT = []
        for m in range(NKIN):
            ps_o = psum.tile([128, SH], F32, tag="mm")
            for c in range(NCI):
                nc.tensor.matmul(ps_o, wout_sb[:, c, m * 128:(m + 1) * 128],
                                 yg[c], start=(c == 0), stop=(c == NCI - 1))
            t = work.tile([128, SH], F32, tag="oT", bufs=NKIN)
            nc.vector.tensor_copy(out=t, in_=ps_o)
            oT.append(t)

        # ---- 10. final transpose + layernorm + residual ----
        ps_fin = psum.tile([SH, DM], F32, tag="fin", bufs=1)
        st2 = work.tile([SH, NKIN, 6], F32)
        for m in range(NKIN):
            nc.tensor.matmul(ps_fin[:, m * 128:(m + 1) * 128], oT[m],
                             id_sb, is_transpose=True, start=True, stop=True)
            nc.vector.bn_stats(out=st2[:, m, :], in_=ps_fin[:, m * 128:(m + 1) * 128])
        mv2 = work.tile([SH, 2], F32)
        nc.vector.bn_aggr(out=mv2, in_=st2)
        rstd2 = _nr_rsqrt(nc, work, mv2[:, 1:2], SH, "r2")
        xhat2 = work.tile([SH, DM], F32)
        nc.vector.tensor_scalar(out=xhat2, in0=ps_fin, scalar1=mv2[:, 0:1],
                                scalar2=rstd2, op0=OP.subtract, op1=OP.mult)
        rb = work.tile([SH, DM], F32)
        nc.vector.tensor_add(rb, bout_rep, xres_sb)
        outf = work.tile([SH, DM], F32)
        nc.vector.tensor_mul(outf, xhat2, gout_rep)
        nc.vector.tensor_add(outf, outf, rb)
        nc.sync.dma_start(out=p_out[:], in_=outf)

    nc.finalize()
    return nc


def _make_in_maps(inputs):
    x = np.asarray(inputs["x"], np.float32)
    A_log = np.asarray(inputs["A_log"], np.float32)
    beta = _fit_beta(A_log)
    rep = np.zeros((SH, 128), np.float32)
    rep[np.arange(128) % SH, np.arange(128)] = 1.0
    ident = np.eye(128, dtype=np.float32)

    if TRANS_DT == F32:
        tnp = np.float32
    elif TRANS_DT == F16:
        tnp = np.float16
    else:
        import ml_dtypes
        tnp = ml_dtypes.bfloat16

    W_in = np.asarray(inputs["W_in"], np.float32)
    g_in = np.asarray(inputs["ln_in_g"], np.float32)
    b_in = np.asarray(inputs["ln_in_b"], np.float32)
    W_in_g = g_in[:, None] * W_in
    bw = (b_in @ W_in).astype(np.float32)

    smalls = np.zeros((128, NSMALL), np.float32)
    cw = np.asarray(inputs["conv_w"], np.float32)[:, 0, :].reshape(NCI, 128, DCONV)
    for c in range(NCI):
        smalls[:, CW0 + 4 * c:CW0 + 4 * c + 4] = cw[c]
    smalls[:, CB0:CB0 + NCI] = np.asarray(inputs["conv_b"], np.float32).reshape(NCI, 128).T
    smalls[:, DD0:DD0 + NCI] = np.asarray(inputs["D"], np.float32).reshape(NCI, 128).T
    smalls[:, DB2_0:DB2_0 + NCI] = np.asarray(inputs["dt_b2"], np.float32).reshape(NCI, 128).T
    smalls[:, DB1_0:DB1_0 + NCH] = np.asarray(inputs["dt_b1"], np.float32).reshape(NCH, 128).T
    smalls[:, BWX0:BWX0 + NCI] = bw[:DI].reshape(NCI, 128).T
    smalls[:, BWZ0:BWZ0 + NCI] = bw[DI:].reshape(NCI, 128).T

    shared = {
        "w_in": _part_rows(W_in_g, NKIN).astype(BIG_NP),
        "w_out": _part_rows(np.asarray(inputs["W_out"], np.float32), NCI).astype(BIG_NP),
        "w_b": _part_rows(np.asarray(inputs["W_B"], np.float32), NCI).astype(np.float16),
        "w_c": _part_rows(np.asarray(inputs["W_C"], np.float32), NCI).astype(np.float16),
        "dt_w1": _part_rows(np.asarray(inputs["dt_w1"], np.float32), NCI).astype(np.float16),
        "dt_w2": _part_rows(0.5 * np.asarray(inputs["dt_w2"], np.float32), NCH).astype(np.float16),
        "smalls": smalls,
        "beta": beta,
        "rep": rep,
        "ident": ident,
        "ident_t": ident.astype(tnp),
    }

    g_out = np.asarray(inputs["ln_out_g"], np.float32)
    b_out = np.asarray(inputs["ln_out_b"], np.float32)
    xf = x[0]
    in_maps = []
    for core in range(NCORES):
        lo = core * SH - HALO
        xs = np.zeros((LH, DM), np.float32)
        mskt = np.zeros(LH, np.float32)
        valid0 = max(0, -lo)
        xs[valid0:] = xf[lo + valid0: lo + LH]
        mskt[valid0:] = 1.0
        reps = np.zeros((128, LH + 2 * DM), np.float32)
        reps[:, :LH] = mskt[None, :]
        reps[:SH, LH:LH + DM] = g_out[None, :]
        reps[:SH, LH + DM:] = b_out[None, :]
        in_maps.append({**shared, "x_sh": xs, "reps": reps})
    return in_maps


def kernel(**inputs):
    if "nc" not in _CACHE:
        _CACHE["nc"] = _build_nc()
    nc = _CACHE["nc"]
    in_maps = _make_in_maps(inputs)
    res = bass_utils.run_bass_kernel_spmd(nc, in_maps, core_ids=list(range(NCORES)))
    out = np.concatenate([res.results[i]["out"] for i in range(NCORES)], axis=0)
    return out.reshape(1, L, DM).astype(np.float32)
